# revision 33
# baseline (speedup 1.0000x reference)
"""Bidirectional Mamba block on 8 Trainium2 NeuronCores (Bass/Tile).

Data-parallel over batch: B=16 -> 2 per core; weights replicated; host gathers.
Per-core layout is feature-major ([feature_partitions, tokens]) with tokens =
batch-major concatenation of the 2 local sequences (t = b*512 + l).

Engines:
  PE   - all projections (weights stationary as lhsT), depthwise causal conv as
         4 accumulating diag-matmuls over shifted views, partition-broadcast of
         per-token B/C rows via one-hot selector matmuls.
  ACT  - exp/ln resident table only: softplus = ln(exp(.)+1), silu via exp,
         rsqrt = exp(-0.5*ln(.)); dA_n = exp(delta * A[:,n]) with per-partition
         scale; fused PSUM->SBUF copies.
  DVE  - selective scan via tensor_tensor_scan (fp32 internal state); the
         backward layer feeds the scan with reversed access patterns.
"""

import numpy as np

# ---- problem constants (hardcoded per contract) ----
B, L, DM = 16, 512, 256
DI, N, R, KC = 512, 16, 16, 4
NCORES = 8
BL = B // NCORES          # local batch
TOK = BL * L              # 1024 tokens per core
DT_TILES = DI // 128      # 4
MT = DM // 128            # 2
F32_np = np.float32

# ---- dtype knobs for the scan path ----
import ml_dtypes
BF16_np = ml_dtypes.bfloat16

CFG = dict(
    DA="bf16",     # dA (scan decay operand)
    DELTA="bf16",  # delta resident
    W="bf16",      # w = delta*xs (scan drive factor)
    H="bf16",      # scan output h
    REP="bf16",    # B_rep / C_rep broadcast tiles
    P="bf16",      # products h*C
    YACC="bf16",   # y accumulator (only the non-PE d-tile)
    SZ="bf16",     # silu(z) gate
    XS="bf16",     # conv-silu output / gate buffer
    POOL_BX=(),       # dt tiles whose bx-mul runs on GPSIMD
    POOL_P=(),        # dt tiles whose p-mul runs on GPSIMD
    ADDS4="dve",      # y_acc3 adds: "dve" or "pool"
    WMUL="dve",       # w = delta*xs engine: "pool" keeps proj off DVE
    FUSE_SCAN=True,   # single full-width scan with dA=0 at sequence starts
    SPLIT_MUL=False,  # split 1024-wide DVE elementwise ops into 512 halves
    SPLIT_SCAN=False, # chain two 256-wide scans per 512 sequence
    DA_BUFS=3,
    SCANW_BUFS=2,
    REP_BUFS=3,
)

_BUILD_CACHE = {}


# ======================================================================
# host-side weight preparation
# ======================================================================

def _prep_layer_weights(inw, convw, convb, xprojw, dtw, dtb, Alog, Dp, outw, normw):
    """Fold/reshape one mamba layer's weights into device layouts."""
    out = {}
    # in_proj with rmsnorm weight folded into rows: [128, 2, 1024]
    w = (np.asarray(normw)[:, None] * np.asarray(inw)).astype(F32_np)
    out["inw"] = np.ascontiguousarray(w.reshape(2, 128, 2 * DI).transpose(1, 0, 2)).astype(BF16_np)
    # conv diag matrices: [128, 16(dt*4+k), 128]
    cd = np.zeros((128, DT_TILES * KC, 128), F32_np)
    cw = np.asarray(convw).astype(F32_np)  # (KC, 1, DI)
    for dt in range(DT_TILES):
        for k in range(KC):
            idx = np.arange(128)
            cd[idx, dt * KC + k, idx] = cw[k, 0, dt * 128 + idx]
    out["convd"] = np.ascontiguousarray(cd).astype(BF16_np)
    out["convb"] = np.ascontiguousarray(
        np.asarray(convb).astype(F32_np).reshape(DT_TILES, 128, 1).transpose(1, 0, 2))
    # xproj padded so delta_raw/B/C land at partitions 0/32/64: [128, 4, 96]
    xp = np.zeros((DI, 96), F32_np)
    xpw = np.asarray(xprojw).astype(F32_np)
    xp[:, 0:R] = xpw[:, 0:R]
    xp[:, 32:32 + N] = xpw[:, R:R + N]
    xp[:, 64:64 + N] = xpw[:, R + N:R + 2 * N]
    out["xpw"] = np.ascontiguousarray(xp.reshape(DT_TILES, 128, 96).transpose(1, 0, 2)).astype(BF16_np)
    out["dtw"] = np.ascontiguousarray(np.asarray(dtw).astype(F32_np)).astype(BF16_np)          # (16, 512)
    out["dtb"] = np.ascontiguousarray(
        np.asarray(dtb).astype(F32_np).reshape(DT_TILES, 128, 1).transpose(1, 0, 2))
    A = (-np.exp(np.asarray(Alog).astype(np.float64))).astype(F32_np)          # (512, 16)
    out["A"] = np.ascontiguousarray(A.reshape(DT_TILES, 128, N).transpose(1, 0, 2))
    out["Dp"] = np.ascontiguousarray(
        np.asarray(Dp).astype(F32_np).reshape(DT_TILES, 128, 1).transpose(1, 0, 2))
    out["outw"] = np.ascontiguousarray(
        np.asarray(outw).astype(F32_np).reshape(DT_TILES, 128, DM).transpose(1, 0, 2)).astype(BF16_np)
    return out


def _prep_shared_weights(proj_w, proj_b, ln_g, ln_b):
    out = {}
    out["projw"] = np.ascontiguousarray(
        np.asarray(proj_w).astype(F32_np).reshape(4, 128, DM).transpose(1, 0, 2)).astype(BF16_np)
    out["projb"] = np.ascontiguousarray(
        np.asarray(proj_b).astype(F32_np).reshape(MT, 128, 1).transpose(1, 0, 2))
    out["lng"] = np.ascontiguousarray(
        np.asarray(ln_g).astype(F32_np).reshape(MT, 128, 1).transpose(1, 0, 2))
    out["lnb"] = np.ascontiguousarray(
        np.asarray(ln_b).astype(F32_np).reshape(MT, 128, 1).transpose(1, 0, 2))
    return out


# ======================================================================
# device program
# ======================================================================

def _build(loop_k=1, cfg=None, variant="full"):
    cfg = dict(CFG if cfg is None else cfg)
    key = (loop_k, variant, tuple(sorted(cfg.items())))
    if key in _BUILD_CACHE:
        return _BUILD_CACHE[key]

    import concourse.bacc as bacc
    import concourse.mybir as mybir
    import concourse.tile as tile

    F32 = mybir.dt.float32
    BF16 = mybir.dt.bfloat16
    AF = mybir.ActivationFunctionType
    ALU = mybir.AluOpType
    AX = mybir.AxisListType

    def dt_of(kname):
        return F32 if cfg[kname] == "f32" else BF16

    nc = bacc.Bacc("TRN2", target_bir_lowering=False, debug=False)

    def din(name, shape, dt=None):
        return nc.dram_tensor(name, list(shape), dt or F32, kind="ExternalInput").ap()

    # --- DRAM I/O ---
    xT_d = din("xT", (DM, TOK))
    lw_d = {}
    for s in ("f", "b"):
        lw_d[s] = {
            "inw": din(f"{s}_inw", (128, 2, 2 * DI), BF16),
            "convd": din(f"{s}_convd", (128, DT_TILES * KC, 128), BF16),
            "convb": din(f"{s}_convb", (128, DT_TILES, 1)),
            "xpw": din(f"{s}_xpw", (128, DT_TILES, 96), BF16),
            "dtw": din(f"{s}_dtw", (16, DI), BF16),
            "dtb": din(f"{s}_dtb", (128, DT_TILES, 1)),
            "A": din(f"{s}_A", (128, DT_TILES, N)),
            "Dp": din(f"{s}_Dp", (128, DT_TILES, 1)),
            "outw": din(f"{s}_outw", (128, DT_TILES, DM), BF16),
        }
    projw_d = din("projw", (128, 4, DM), BF16)
    projb_d = din("projb", (128, MT, 1))
    lng_d = din("lng", (128, MT, 1))
    lnb_d = din("lnb", (128, MT, 1))
    outT_d = nc.dram_tensor("outT", [DM, TOK], F32, kind="ExternalOutput").ap()

    PAD = KC - 1  # 3
    CONVW = 2 * PAD + L  # padded per-batch row length 518

    with tile.TileContext(nc) as tc:
        from contextlib import ExitStack
        with ExitStack() as ctx:
            wpool = ctx.enter_context(tc.tile_pool(name="wpool", bufs=1))
            pers = ctx.enter_context(tc.tile_pool(name="pers", bufs=1))
            work = ctx.enter_context(tc.tile_pool(name="work", bufs=2))
            rep = ctx.enter_context(tc.tile_pool(name="rep", bufs=2))
            scanw = ctx.enter_context(tc.tile_pool(name="scanw", bufs=2))

            def body():
                # ---- load shared weights ----
                projw_t = wpool.tile([128, 4, DM], BF16, tag="projw", name="projw")
                nc.sync.dma_start(projw_t[:], projw_d[:])
                projb_t = wpool.tile([128, MT, 1], F32, tag="projb", name="projb")
                nc.sync.dma_start(projb_t[:], projb_d[:])
                lng_t = wpool.tile([128, MT, 1], F32, tag="lng", name="lng")
                nc.sync.dma_start(lng_t[:], lng_d[:])
                lnb_t = wpool.tile([128, MT, 1], F32, tag="lnb", name="lnb")
                nc.sync.dma_start(lnb_t[:], lnb_d[:])

                xtp_ctx = tc.tile_pool(name="xtp", bufs=1)
                xtp = xtp_ctx.__enter__()
                xT = []
                for m in range(MT):
                    t = xtp.tile([128, TOK], F32, tag=f"xT{m}", name=f"xT{m}")
                    nc.sync.dma_start(t[:], xT_d[m * 128:(m + 1) * 128, :])
                    xT.append(t)

                # ---- shared RMSNorm: xn = x * rsqrt(mean(x^2) + eps) ----
                xn = []
                with tc.tile_pool(name="prms", bufs=1, space="PSUM") as prms:
                    ones_col = wpool.tile([128, 1], F32, tag="ones_col", name="ones_col")
                    nc.vector.memset(ones_col[:], 1.0)
                    ss_ps = prms.tile([1, TOK], F32, tag="ss", name="ss")
                    for fh in range(2):
                        fs = slice(fh * 512, (fh + 1) * 512)
                        for m in range(MT):
                            sq = work.tile([128, 512], F32, tag="sqtmp", name="rms_sq")
                            nc.scalar.square(sq[:], xT[m][:, fs])
                            nc.tensor.matmul(ss_ps[:, fs],ones_col[:],sq[:],
                                             start=(m == 0), stop=(m == MT - 1))
                    # rs = exp(-0.5 * ln(ss/DM + eps))
                    eps1 = wpool.tile([1, 1], F32, tag="eps1", name="eps1")
                    nc.vector.memset(eps1[:], 1e-5)
                    rs_row = work.tile([1, TOK], F32, tag="rowtmp", name="rs_row", bufs=1)
                    nc.scalar.activation(rs_row[:], ss_ps[:], AF.Ln,
                                         scale=1.0 / DM, bias=eps1[:, 0:1])
                    nc.scalar.activation(rs_row[:], rs_row[:], AF.Exp, scale=-0.5)
                    ones1 = wpool.tile([1, 128], F32, tag="ones1", name="ones1")
                    nc.vector.memset(ones1[:], 1.0)
                    rs_ps = prms.tile([128, TOK], F32, tag="rs_rep", name="rs_rep")
                    for fh in range(2):
                        fs = slice(fh * 512, (fh + 1) * 512)
                        nc.tensor.matmul(rs_ps[:, fs],ones1[:],rs_row[:, fs],
                                         start=True, stop=True)
                    for m in range(MT):
                        t = pers.tile([128, TOK], BF16, tag=f"xn{m}", name=f"xn{m}")
                        nc.vector.tensor_mul(t[:], xT[m][:], rs_ps[:])
                        xn.append(t)

                # bf16 copy of x for PE-side residual accumulation
                xTb = []
                for m in range(MT):
                    t = pers.tile([128, TOK], BF16, tag=f"xTb{m}", name=f"xTb{m}")
                    nc.scalar.copy(t[:], xT[m][:])
                    xTb.append(t)
                xtp_ctx.__exit__(None, None, None)

                # ---- per-layer DRAM scratch for B/C row broadcast ----
                dbc_d = {}
                for s2 in ("f", "b"):
                    dbc_d[s2] = nc.dram_tensor(f"{s2}_dbc_scratch", [16, 2, TOK],
                                               BF16, kind="Internal").ap()

                idn = wpool.tile([128, 128], BF16, tag="idn", name="idn")
                from concourse.masks import make_identity
                make_identity(nc, idn[:])

                N_PE_DT = 3

                # ---- proj stage A: in_proj, conv, z-gate (silu table) ----
                def proj_a(s, reverse, ppb):
                    W = lw_d[s]
                    inw_t = wpool.tile([128, 2, 2 * DI], BF16, tag=f"inw{s}", name=f"inw{s}")
                    nc.sync.dma_start(inw_t[:], W["inw"][:])
                    convd_t = wpool.tile([128, DT_TILES * KC, 128], BF16, tag=f"convd{s}", name=f"convd{s}")
                    nc.sync.dma_start(convd_t[:], W["convd"][:])
                    convb_t = wpool.tile([128, DT_TILES, 1], F32, tag=f"convb{s}", name=f"convb{s}")
                    nc.sync.dma_start(convb_t[:], W["convb"][:])

                    xmpad = []
                    sz = []
                    xs = []
                    for dt in range(DT_TILES):
                        t = pers.tile([128, BL, CONVW], BF16, tag=f"xmpad{dt}", name=f"xmpad{dt}")
                        nc.gpsimd.memset(t[:, :, 0:PAD], 0.0)
                        nc.gpsimd.memset(t[:, :, PAD + L:CONVW], 0.0)
                        xmpad.append(t)
                        sz.append(pers.tile([128, TOK], dt_of("SZ"), tag=f"sz{s}{dt}", name=f"sz{s}{dt}"))
                        xs.append(pers.tile([128, TOK], dt_of("XS"), tag=f"xs{s}{dt}", name=f"xs{s}{dt}"))

                    # in_proj: xm tiles first (conv depends on them), z last.
                    # Full-width PSUM tiles: fewer, larger PE/ACT ops (both
                    # proj_a phases run before pyac claims its banks).
                    with tc.tile_pool(name="pp", bufs=ppb, space="PSUM") as pp:
                        for m in range(4):
                            ps = pp.tile([128, TOK], F32, tag="ppw", name="ppw", bufs=2)
                            for fh in range(2):
                                fs = slice(fh * 512, (fh + 1) * 512)
                                for ks in range(2):
                                    nc.tensor.matmul(
                                        ps[:, fs], inw_t[:, ks, m * 128:(m + 1) * 128],
                                        xn[ks][:, fs], start=(ks == 0), stop=(ks == 1))
                            # both halves -> padded conv buffer in one copy
                            nc.scalar.copy(xmpad[m][:, :, PAD:PAD + L], ps[:])

                        # depthwise causal conv + silu
                        for dt in range(DT_TILES):
                            for b in range(BL):
                                ps = pp.tile([128, 512], F32, tag="pp", name="pp")
                                for k in range(KC):
                                    off = k if not reverse else (2 * PAD - k)
                                    nc.tensor.matmul(
                                        ps[:], convd_t[:, dt * KC + k, :],
                                        xmpad[dt][:, b, off:off + L],
                                        start=(k == 0), stop=(k == KC - 1))
                                bs = slice(b * L, (b + 1) * L)
                                nc.scalar.activation(xs[dt][:, bs], ps[:], AF.Silu,
                                                     bias=convb_t[:, dt, 0:1])

                        # z gate tiles (not on the critical chain to the scan)
                        for m in range(4, 8):
                            ps = pp.tile([128, TOK], F32, tag="ppw", name="ppw", bufs=2)
                            for fh in range(2):
                                fs = slice(fh * 512, (fh + 1) * 512)
                                for ks in range(2):
                                    nc.tensor.matmul(
                                        ps[:, fs], inw_t[:, ks, m * 128:(m + 1) * 128],
                                        xn[ks][:, fs], start=(ks == 0), stop=(ks == 1))
                            nc.scalar.activation(sz[m - 4][:], ps[:], AF.Silu)
                    return dict(xs=xs, sz=sz)

                # ---- proj stage B: xproj + dt_proj (exp/ln table only) ----
                # Generator: yields between chunks so it can be interleaved
                # into the other layer's scan loop.
                def proj_b_gen(s, reverse, st):
                    W = lw_d[s]
                    xpw_t = wpool.tile([128, DT_TILES, 96], BF16, tag=f"xpw{s}", name=f"xpw{s}")
                    nc.sync.dma_start(xpw_t[:], W["xpw"][:])
                    dtw_t = wpool.tile([16, DI], BF16, tag=f"dtw{s}", name=f"dtw{s}")
                    nc.sync.dma_start(dtw_t[:], W["dtw"][:])
                    dtb_t = wpool.tile([128, DT_TILES, 1], F32, tag=f"dtb{s}", name=f"dtb{s}")
                    nc.sync.dma_start(dtb_t[:], W["dtb"][:])
                    xs = st["xs"]

                    # xproj -> delta_raw / Brows / Crows; rows to DRAM for bcast
                    dbc = pers.tile([16, 2, TOK], BF16, tag="dbc", name=f"dbc{s}")
                    draw_t = pers.tile([16, TOK], BF16, tag="draw", name=f"draw{s}")
                    draw = draw_t[:, :]
                    with tc.tile_pool(name="pxp", bufs=1, space="PSUM") as pxp:
                        psx = pxp.tile([96, TOK], F32, tag="pxp", name="pxp")
                        for fh in range(2):
                            fs = slice(fh * 512, (fh + 1) * 512)
                            for ks in range(DT_TILES):
                                nc.tensor.matmul(psx[:, fs], xpw_t[:, ks, :], xs[ks][:, fs],
                                                 start=(ks == 0), stop=(ks == DT_TILES - 1))
                            yield
                        nc.scalar.copy(draw, psx[0:16, :])
                        nc.scalar.copy(dbc[:, 0, :], psx[32:48, :])
                        nc.scalar.copy(dbc[:, 1, :], psx[64:80, :])
                    nc.sync.dma_start(dbc_d[s][:], dbc[:])
                    yield

                    # dt_proj + softplus -> delta; w = delta * xs
                    delta = []
                    w_t = []
                    with tc.tile_pool(name="pdt", bufs=2, space="PSUM") as pdt:
                        for dt in range(DT_TILES):
                            dl = pers.tile([128, TOK], dt_of("DELTA"), tag=f"delta{s}{dt}", name=f"delta{s}{dt}")
                            for fh in range(2):
                                fs = slice(fh * 512, (fh + 1) * 512)
                                ps = pdt.tile([128, 512], F32, tag="pdt", name="pdt")
                                nc.tensor.matmul(ps[:], dtw_t[:, dt * 128:(dt + 1) * 128],
                                                 draw[:, fs], start=True, stop=True)
                                e = work.tile([128, 512], F32, tag="detag", name="de")
                                nc.scalar.activation(e[:], ps[:], AF.Exp,
                                                     bias=dtb_t[:, dt, 0:1])
                                nc.scalar.activation(dl[:, fs], e[:], AF.Ln, bias=1.0)
                            delta.append(dl)
                            wt = pers.tile([128, TOK], dt_of("W"), tag=f"w{s}{dt}", name=f"w{s}{dt}")
                            if cfg["WMUL"] == "pool":
                                nc.gpsimd.tensor_mul(wt[:], dl[:], xs[dt][:])
                            else:
                                nc.vector.tensor_mul(wt[:], dl[:], xs[dt][:])
                            w_t.append(wt)
                            if cfg["FUSE_SCAN"]:
                                # poison delta at each sequence start (in scan
                                # order) so exp(delta*A) underflows to zero and
                                # the state resets inside one full-width scan.
                                # (A < 0 strictly, so +1e30 * A -> exp -> 0.)
                                for zc in ((L, 0) if not reverse else (L - 1, 2 * L - 1)):
                                    nc.gpsimd.memset(dl[:, zc:zc + 1], 1e30)
                            yield
                    st["delta"] = delta
                    st["w"] = w_t

                # ---- selective scan + gate + out_proj ----
                def scan_phase(s, reverse, st, y_ps, y_ps3=None, co=None):
                    W = lw_d[s]
                    A_t = wpool.tile([128, DT_TILES, N], F32, tag=f"A{s}", name=f"A{s}")
                    nc.sync.dma_start(A_t[:], W["A"][:])
                    Dp_t = wpool.tile([128, DT_TILES, 1], F32, tag=f"Dp{s}", name=f"Dp{s}")
                    nc.sync.dma_start(Dp_t[:], W["Dp"][:])
                    outw_t = wpool.tile([128, DT_TILES, DM], BF16, tag=f"outw{s}", name=f"outw{s}")
                    nc.sync.dma_start(outw_t[:], W["outw"][:])
                    xs, sz, delta, w_t = st["xs"], st["sz"], st["delta"], st["w"]

                    y_acc3 = pers.tile([128, TOK], dt_of("YACC"), tag=f"yacc{s}", name=f"yacc{s}")
                    pool_bx = cfg["POOL_BX"]
                    pool_p = cfg["POOL_P"]
                    sbufs = cfg["SCANW_BUFS"]
                    n_pe = N_PE_DT if y_ps3 is None else DT_TILES
                    for n in range(N):
                        if co is not None:
                            next(co, None)
                        B_t = rep.tile([128, TOK], dt_of("REP"), tag="B_rep", name="B_rep", bufs=cfg["REP_BUFS"])
                        nc.sync.dma_start(B_t[:], dbc_d[s][n:n + 1, 0, :].broadcast_to([128, TOK]))
                        C_t = rep.tile([128, TOK], dt_of("REP"), tag="C_rep", name="C_rep", bufs=cfg["REP_BUFS"])
                        # C on the ACT-triggered HWDGE queue: halves per-queue load
                        nc.scalar.dma_start(C_t[:], dbc_d[s][n:n + 1, 1, :].broadcast_to([128, TOK]))
                        B_rep = B_t[:, :]
                        C_rep = C_t[:, :]
                        for dt in range(DT_TILES):
                            dA = scanw.tile([128, TOK], dt_of("DA"), tag="dA", name="dA", bufs=cfg["DA_BUFS"])
                            nc.scalar.activation(dA[:], delta[dt][:], AF.Exp,
                                                 scale=A_t[:, dt, n:n + 1])
                            bx = scanw.tile([128, TOK], dt_of("W"), tag="bx", name="bx", bufs=sbufs)
                            if dt in pool_bx:
                                nc.gpsimd.tensor_mul(bx[:], w_t[dt][:], B_rep)
                            elif cfg["SPLIT_MUL"]:
                                for fh in range(2):
                                    fs = slice(fh * 512, (fh + 1) * 512)
                                    nc.vector.tensor_mul(bx[:, fs], w_t[dt][:, fs], B_rep[:, fs])
                            else:
                                nc.vector.tensor_mul(bx[:], w_t[dt][:], B_rep)
                            h = scanw.tile([128, TOK], dt_of("H"), tag="h", name="h", bufs=sbufs)
                            if cfg["FUSE_SCAN"]:
                                if not reverse:
                                    nc.vector.tensor_tensor_scan(
                                        h[:, :], dA[:, :], bx[:, :], 0.0,
                                        ALU.mult, ALU.add)
                                else:
                                    nc.vector.tensor_tensor_scan(
                                        h[:, :][:, ::-1], dA[:, :][:, ::-1],
                                        bx[:, :][:, ::-1], 0.0,
                                        ALU.mult, ALU.add)
                            elif cfg["SPLIT_SCAN"]:
                                HL = L // 2
                                for b in range(BL):
                                    o = b * L
                                    c0 = slice(o, o + HL)
                                    c1 = slice(o + HL, o + L)
                                    if not reverse:
                                        nc.vector.tensor_tensor_scan(
                                            h[:, c0], dA[:, c0], bx[:, c0], 0.0,
                                            ALU.mult, ALU.add)
                                        nc.vector.tensor_tensor_scan(
                                            h[:, c1], dA[:, c1], bx[:, c1],
                                            h[:, o + HL - 1:o + HL],
                                            ALU.mult, ALU.add)
                                    else:
                                        nc.vector.tensor_tensor_scan(
                                            h[:, c1][:, ::-1], dA[:, c1][:, ::-1],
                                            bx[:, c1][:, ::-1], 0.0,
                                            ALU.mult, ALU.add)
                                        nc.vector.tensor_tensor_scan(
                                            h[:, c0][:, ::-1], dA[:, c0][:, ::-1],
                                            bx[:, c0][:, ::-1],
                                            h[:, o + HL:o + HL + 1],
                                            ALU.mult, ALU.add)
                            else:
                                for b in range(BL):
                                    bs = slice(b * L, (b + 1) * L)
                                    if not reverse:
                                        nc.vector.tensor_tensor_scan(
                                            h[:, bs], dA[:, bs], bx[:, bs], 0.0,
                                            ALU.mult, ALU.add)
                                    else:
                                        # reversed-time scan, output written back
                                        # in forward token order
                                        nc.vector.tensor_tensor_scan(
                                            h[:, bs][:, ::-1], dA[:, bs][:, ::-1],
                                            bx[:, bs][:, ::-1], 0.0,
                                            ALU.mult, ALU.add)
                            if dt < n_pe or n > 0:
                                p = scanw.tile([128, TOK], dt_of("P"), tag="p", name="p", bufs=sbufs)
                                if dt in pool_p:
                                    nc.gpsimd.tensor_mul(p[:], h[:], C_rep)
                                elif cfg["SPLIT_MUL"]:
                                    for fh in range(2):
                                        fs = slice(fh * 512, (fh + 1) * 512)
                                        nc.vector.tensor_mul(p[:, fs], h[:, fs], C_rep[:, fs])
                                else:
                                    nc.vector.tensor_mul(p[:], h[:], C_rep)
                                if dt < n_pe:
                                    yp = y_ps[dt] if dt < N_PE_DT else y_ps3
                                    for fh in range(2):
                                        fs = slice(fh * 512, (fh + 1) * 512)
                                        nc.tensor.matmul(yp[:, fs], idn[:], p[:, fs],
                                                         start=(n == 0), stop=(n == N - 1))
                                elif cfg["ADDS4"] == "pool":
                                    nc.gpsimd.tensor_add(y_acc3[:], y_acc3[:], p[:])
                                elif cfg["SPLIT_MUL"]:
                                    for fh in range(2):
                                        fs = slice(fh * 512, (fh + 1) * 512)
                                        nc.vector.tensor_add(y_acc3[:, fs], y_acc3[:, fs], p[:, fs])
                                else:
                                    nc.vector.tensor_add(y_acc3[:], y_acc3[:], p[:])
                            else:
                                # dt == 3, n == 0: initialize y_acc3 = h * C
                                if cfg["SPLIT_MUL"]:
                                    for fh in range(2):
                                        fs = slice(fh * 512, (fh + 1) * 512)
                                        nc.vector.tensor_mul(y_acc3[:, fs], h[:, fs], C_rep[:, fs])
                                else:
                                    nc.vector.tensor_mul(y_acc3[:], h[:], C_rep)

                    # gate: y = y_acc + Dp*xs, then * silu(z) -- in place on xs
                    g = xs
                    for dt in range(DT_TILES):
                        ysrc = (y_ps[dt] if dt < N_PE_DT else
                                (y_ps3 if y_ps3 is not None else y_acc3))
                        if cfg["SPLIT_MUL"]:
                            for fh in range(2):
                                fs = slice(fh * 512, (fh + 1) * 512)
                                nc.vector.scalar_tensor_tensor(
                                    xs[dt][:, fs], xs[dt][:, fs], Dp_t[:, dt, 0:1],
                                    ysrc[:, fs], ALU.mult, ALU.add)
                                nc.vector.tensor_mul(xs[dt][:, fs], xs[dt][:, fs], sz[dt][:, fs])
                        else:
                            nc.vector.scalar_tensor_tensor(
                                xs[dt][:], xs[dt][:], Dp_t[:, dt, 0:1], ysrc[:],
                                ALU.mult, ALU.add)
                            nc.vector.tensor_mul(xs[dt][:], xs[dt][:], sz[dt][:])
                    return g, outw_t

                # ---- out_proj + residual (residual accumulated on PE) ----
                def out_proj(s, g, outw_t):
                    xout = []
                    with tc.tile_pool(name="po", bufs=2, space="PSUM") as po:
                        for m in range(MT):
                            t = pers.tile([128, TOK], BF16, tag=f"x{s}out{m}", name=f"x{s}out{m}")
                            for fh in range(2):
                                fs = slice(fh * 512, (fh + 1) * 512)
                                ps = po.tile([128, 512], F32, tag="po", name="po")
                                for ks in range(DT_TILES):
                                    nc.tensor.matmul(
                                        ps[:], outw_t[:, ks, m * 128:(m + 1) * 128],
                                        g[ks][:, fs], start=(ks == 0), stop=False)
                                nc.tensor.matmul(ps[:], idn[:], xTb[m][:, fs],
                                                 start=False, stop=True)
                                nc.scalar.copy(t[:, fs], ps[:])
                            xout.append(t)
                    return xout

                stf = proj_a("f", reverse=False, ppb=4)
                stb = proj_a("b", reverse=True, ppb=4)
                # scan-phase y accumulators claim 6 PSUM banks; the interleaved
                # stage-B pools for layer b fit in the remaining 2.
                pyac_ctx = tc.tile_pool(name="pyac", bufs=1, space="PSUM")
                pyac = pyac_ctx.__enter__()
                y_ps = [pyac.tile([128, TOK], F32, tag=f"yps{dt}", name=f"yps{dt}")
                        for dt in range(N_PE_DT)]
                for _ in proj_b_gen("f", reverse=False, st=stf):
                    pass
                cob = proj_b_gen("b", reverse=True, st=stb)
                g1, outw_f = scan_phase("f", reverse=False, st=stf, y_ps=y_ps, co=cob)
                for _ in cob:   # drain any remaining stage-B work of layer b
                    pass
                x1 = out_proj("f", g1, outw_f)
                # layer b scan runs with all four d-tiles accumulated on PE:
                # the proj pools are closed by now, freeing the last 2 banks.
                pyac2_ctx = tc.tile_pool(name="pyac2", bufs=1, space="PSUM")
                pyac2 = pyac2_ctx.__enter__()
                y_ps3 = pyac2.tile([128, TOK], F32, tag="yps3", name="yps3")
                g2, outw_b = scan_phase("b", reverse=True, st=stb, y_ps=y_ps, y_ps3=y_ps3)
                pyac2_ctx.__exit__(None, None, None)
                pyac_ctx.__exit__(None, None, None)
                x2 = out_proj("b", g2, outw_b)

                # ---- head: relu(cat(x1,x2) @ proj_w + proj_b), residual, layernorm ----
                cat = x1 + x2
                xn2 = []
                with tc.tile_pool(name="ph", bufs=3, space="PSUM") as ph:
                    for m in range(MT):
                        x2n = pers.tile([128, TOK], F32, tag=f"xn2_{m}", name=f"xn2_{m}")
                        for fh in range(2):
                            fs = slice(fh * 512, (fh + 1) * 512)
                            ps = ph.tile([128, 512], F32, tag="ph", name="ph")
                            for ks in range(4):
                                nc.tensor.matmul(
                                    ps[:],projw_t[:, ks, m * 128:(m + 1) * 128],cat[ks][:, fs], start=(ks == 0), stop=(ks == 3))
                            t = work.tile([128, 512], F32, tag="yh", name="yh")
                            nc.scalar.activation(t[:], ps[:], AF.Relu,
                                                 bias=projb_t[:, m, 0:1])
                            nc.vector.tensor_add(x2n[:, fs], t[:], xTb[m][:, fs])
                        xn2.append(x2n)

                with tc.tile_pool(name="pln", bufs=1, space="PSUM") as pln:
                    ones_col = wpool.tile([128, 1], F32, tag="ones_col2", name="ones_col2")
                    nc.vector.memset(ones_col[:], 1.0)
                    ones1 = wpool.tile([1, 128], F32, tag="ones1b", name="ones1b")
                    nc.vector.memset(ones1[:], 1.0)
                    mu_ps = pln.tile([1, TOK], F32, tag="mu", name="mu")
                    ss_ps = pln.tile([1, TOK], F32, tag="ss2", name="ss2")
                    for fh in range(2):
                        fs = slice(fh * 512, (fh + 1) * 512)
                        for m in range(MT):
                            nc.tensor.matmul(mu_ps[:, fs],ones_col[:],xn2[m][:, fs],
                                             start=(m == 0), stop=(m == MT - 1))
                            sq = work.tile([128, 512], F32, tag="sqtmp", name="ln_sq")
                            nc.scalar.square(sq[:], xn2[m][:, fs])
                            nc.tensor.matmul(ss_ps[:, fs],ones_col[:],sq[:],
                                             start=(m == 0), stop=(m == MT - 1))
                    mu_row = wpool.tile([1, TOK], F32, tag="mu_row", name="mu_row")
                    nc.scalar.mul(mu_row[:], mu_ps[:], 1.0 / DM)
                    # var = ss/DM - mu^2 (built in rstd_row, then rstd in place)
                    rstd_row = wpool.tile([1, TOK], F32, tag="rstd_row", name="rstd_row")
                    nc.scalar.mul(rstd_row[:], ss_ps[:], 1.0 / DM)
                    mu2 = work.tile([1, TOK], F32, tag="rowtmp", name="mu2", bufs=1)
                    nc.vector.tensor_mul(mu2[:], mu_row[:], mu_row[:])
                    nc.vector.tensor_sub(rstd_row[:], rstd_row[:], mu2[:])
                    eps2 = wpool.tile([1, 1], F32, tag="eps2", name="eps2")
                    nc.vector.memset(eps2[:], 1e-5)
                    nc.scalar.activation(rstd_row[:], rstd_row[:], AF.Ln, bias=eps2[:, 0:1])
                    nc.scalar.activation(rstd_row[:], rstd_row[:], AF.Exp, scale=-0.5)
                    mu_rep = pln.tile([128, TOK], F32, tag="mu_rep", name="mu_rep")
                    rs_rep = pln.tile([128, TOK], F32, tag="rs_rep2", name="rs_rep2")
                    for fh in range(2):
                        fs = slice(fh * 512, (fh + 1) * 512)
                        nc.tensor.matmul(mu_rep[:, fs],ones1[:],mu_row[:, fs],
                                         start=True, stop=True)
                        nc.tensor.matmul(rs_rep[:, fs],ones1[:],rstd_row[:, fs],
                                         start=True, stop=True)
                    for m in range(MT):
                        nc.vector.tensor_sub(xn2[m][:], xn2[m][:], mu_rep[:])
                        nc.vector.tensor_mul(xn2[m][:], xn2[m][:], rs_rep[:])
                        nc.scalar.activation(xn2[m][:], xn2[m][:], AF.Identity,
                                             bias=lnb_t[:, m, 0:1],
                                             scale=lng_t[:, m, 0:1])
                        nc.sync.dma_start(outT_d[m * 128:(m + 1) * 128, :], xn2[m][:])

            if loop_k > 1:
                with tc.For_i(0, loop_k, 1):
                    body()
            else:
                body()

    nc.compile()
    _BUILD_CACHE[key] = nc
    return nc


# ======================================================================
# host entry
# ======================================================================

def _make_in_maps(inputs):
    x = np.asarray(inputs["x"], F32_np)
    fw = _prep_layer_weights(inputs["fm_in"], inputs["fm_convw"], inputs["fm_convb"],
                             inputs["fm_xproj"], inputs["fm_dtw"], inputs["fm_dtb"],
                             inputs["fm_Alog"], inputs["fm_D"], inputs["fm_out"],
                             inputs["fm_norm"])
    bw = _prep_layer_weights(inputs["bm_in"], inputs["bm_convw"], inputs["bm_convb"],
                             inputs["bm_xproj"], inputs["bm_dtw"], inputs["bm_dtb"],
                             inputs["bm_Alog"], inputs["bm_D"], inputs["bm_out"],
                             inputs["bm_norm"])
    sh = _prep_shared_weights(inputs["proj_w"], inputs["proj_b"],
                              inputs["ln_g"], inputs["ln_b"])
    base = {}
    for s, w in (("f", fw), ("b", bw)):
        for k, v in w.items():
            if k in ("convb", "inw", "convd", "xpw", "dtw", "dtb", "A", "Dp",
                     "outw"):
                base[f"{s}_{k}"] = v
    base["projw"] = sh["projw"]
    base["projb"] = sh["projb"]
    base["lng"] = sh["lng"]
    base["lnb"] = sh["lnb"]

    in_maps = []
    for c in range(NCORES):
        xc = x[c * BL:(c + 1) * BL]                       # (BL, L, DM)
        xTc = np.ascontiguousarray(xc.reshape(TOK, DM).T)  # (DM, TOK)
        m = dict(base)
        m["xT"] = xTc
        in_maps.append(m)
    return in_maps


def _unshard(results):
    outs = []
    for c in range(NCORES):
        oT = results[c]["outT"]                            # (DM, TOK)
        outs.append(np.ascontiguousarray(oT.T.reshape(BL, L, DM)))
    return np.concatenate(outs, axis=0).astype(F32_np)


def kernel(**inputs):
    from concourse import bass_utils
    nc = _build(loop_k=1)
    in_maps = _make_in_maps(inputs)
    res = bass_utils.run_bass_kernel_spmd(nc, in_maps, core_ids=list(range(NCORES)))
    return _unshard(res.results)



# revision 34
# speedup vs baseline: 1.0058x; 1.0058x over previous
"""Bidirectional Mamba block on 8 Trainium2 NeuronCores (Bass/Tile).

Data-parallel over batch: B=16 -> 2 per core; weights replicated; host gathers.
Per-core layout is feature-major ([feature_partitions, tokens]) with tokens =
batch-major concatenation of the 2 local sequences (t = b*512 + l).

Engines:
  PE   - all projections (weights stationary as lhsT), depthwise causal conv as
         4 accumulating diag-matmuls over shifted views, partition-broadcast of
         per-token B/C rows via one-hot selector matmuls.
  ACT  - exp/ln resident table only: softplus = ln(exp(.)+1), silu via exp,
         rsqrt = exp(-0.5*ln(.)); dA_n = exp(delta * A[:,n]) with per-partition
         scale; fused PSUM->SBUF copies.
  DVE  - selective scan via tensor_tensor_scan (fp32 internal state); the
         backward layer feeds the scan with reversed access patterns.
"""

import numpy as np

# ---- problem constants (hardcoded per contract) ----
B, L, DM = 16, 512, 256
DI, N, R, KC = 512, 16, 16, 4
NCORES = 8
BL = B // NCORES          # local batch
TOK = BL * L              # 1024 tokens per core
DT_TILES = DI // 128      # 4
MT = DM // 128            # 2
F32_np = np.float32

# ---- dtype knobs for the scan path ----
import ml_dtypes
BF16_np = ml_dtypes.bfloat16

CFG = dict(
    DA="bf16",     # dA (scan decay operand)
    DELTA="bf16",  # delta resident
    W="bf16",      # w = delta*xs (scan drive factor)
    H="bf16",      # scan output h
    REP="bf16",    # B_rep / C_rep broadcast tiles
    P="bf16",      # products h*C
    YACC="bf16",   # y accumulator (only the non-PE d-tile)
    SZ="bf16",     # silu(z) gate
    XS="bf16",     # conv-silu output / gate buffer
    POOL_BX=(),       # dt tiles whose bx-mul runs on GPSIMD
    POOL_P=(),        # dt tiles whose p-mul runs on GPSIMD
    ADDS4="dve",      # y_acc3 adds: "dve" or "pool"
    WMUL="dve",       # w = delta*xs engine: "pool" keeps proj off DVE
    FUSE_SCAN=True,   # single full-width scan with dA=0 at sequence starts
    SPLIT_MUL=False,  # split 1024-wide DVE elementwise ops into 512 halves
    SPLIT_SCAN=False, # chain two 256-wide scans per 512 sequence
    DA_BUFS=3,
    SCANW_BUFS=2,
    REP_BUFS=3,
)

_BUILD_CACHE = {}


# ======================================================================
# host-side weight preparation
# ======================================================================

def _prep_layer_weights(inw, convw, convb, xprojw, dtw, dtb, Alog, Dp, outw, normw):
    """Fold/reshape one mamba layer's weights into device layouts."""
    out = {}
    # in_proj with rmsnorm weight folded into rows: [128, 2, 1024]
    w = (np.asarray(normw)[:, None] * np.asarray(inw)).astype(F32_np)
    out["inw"] = np.ascontiguousarray(w.reshape(2, 128, 2 * DI).transpose(1, 0, 2)).astype(BF16_np)
    # conv diag matrices: [128, 16(dt*4+k), 128]
    cd = np.zeros((128, DT_TILES * KC, 128), F32_np)
    cw = np.asarray(convw).astype(F32_np)  # (KC, 1, DI)
    for dt in range(DT_TILES):
        for k in range(KC):
            idx = np.arange(128)
            cd[idx, dt * KC + k, idx] = cw[k, 0, dt * 128 + idx]
    out["convd"] = np.ascontiguousarray(cd).astype(BF16_np)
    out["convb"] = np.ascontiguousarray(
        np.asarray(convb).astype(F32_np).reshape(DT_TILES, 128, 1).transpose(1, 0, 2))
    # xproj padded so delta_raw/B/C land at partitions 0/32/64: [128, 4, 96]
    xp = np.zeros((DI, 96), F32_np)
    xpw = np.asarray(xprojw).astype(F32_np)
    xp[:, 0:R] = xpw[:, 0:R]
    xp[:, 32:32 + N] = xpw[:, R:R + N]
    xp[:, 64:64 + N] = xpw[:, R + N:R + 2 * N]
    out["xpw"] = np.ascontiguousarray(xp.reshape(DT_TILES, 128, 96).transpose(1, 0, 2)).astype(BF16_np)
    out["dtw"] = np.ascontiguousarray(np.asarray(dtw).astype(F32_np)).astype(BF16_np)          # (16, 512)
    out["dtb"] = np.ascontiguousarray(
        np.asarray(dtb).astype(F32_np).reshape(DT_TILES, 128, 1).transpose(1, 0, 2))
    A = (-np.exp(np.asarray(Alog).astype(np.float64))).astype(F32_np)          # (512, 16)
    out["A"] = np.ascontiguousarray(A.reshape(DT_TILES, 128, N).transpose(1, 0, 2))
    out["Dp"] = np.ascontiguousarray(
        np.asarray(Dp).astype(F32_np).reshape(DT_TILES, 128, 1).transpose(1, 0, 2))
    out["outw"] = np.ascontiguousarray(
        np.asarray(outw).astype(F32_np).reshape(DT_TILES, 128, DM).transpose(1, 0, 2)).astype(BF16_np)
    return out


def _prep_shared_weights(proj_w, proj_b, ln_g, ln_b):
    out = {}
    out["projw"] = np.ascontiguousarray(
        np.asarray(proj_w).astype(F32_np).reshape(4, 128, DM).transpose(1, 0, 2)).astype(BF16_np)
    out["projb"] = np.ascontiguousarray(
        np.asarray(proj_b).astype(F32_np).reshape(MT, 128, 1).transpose(1, 0, 2))
    out["lng"] = np.ascontiguousarray(
        np.asarray(ln_g).astype(F32_np).reshape(MT, 128, 1).transpose(1, 0, 2))
    out["lnb"] = np.ascontiguousarray(
        np.asarray(ln_b).astype(F32_np).reshape(MT, 128, 1).transpose(1, 0, 2))
    return out


# ======================================================================
# device program
# ======================================================================

def _build(loop_k=1, cfg=None, variant="full"):
    cfg = dict(CFG if cfg is None else cfg)
    key = (loop_k, variant, tuple(sorted(cfg.items())))
    if key in _BUILD_CACHE:
        return _BUILD_CACHE[key]

    import concourse.bacc as bacc
    import concourse.mybir as mybir
    import concourse.tile as tile

    F32 = mybir.dt.float32
    BF16 = mybir.dt.bfloat16
    AF = mybir.ActivationFunctionType
    ALU = mybir.AluOpType
    AX = mybir.AxisListType

    def dt_of(kname):
        return F32 if cfg[kname] == "f32" else BF16

    nc = bacc.Bacc("TRN2", target_bir_lowering=False, debug=False)

    def din(name, shape, dt=None):
        return nc.dram_tensor(name, list(shape), dt or F32, kind="ExternalInput").ap()

    # --- DRAM I/O ---
    xT_d = din("xT", (DM, TOK))
    lw_d = {}
    for s in ("f", "b"):
        lw_d[s] = {
            "inw": din(f"{s}_inw", (128, 2, 2 * DI), BF16),
            "convd": din(f"{s}_convd", (128, DT_TILES * KC, 128), BF16),
            "convb": din(f"{s}_convb", (128, DT_TILES, 1)),
            "xpw": din(f"{s}_xpw", (128, DT_TILES, 96), BF16),
            "dtw": din(f"{s}_dtw", (16, DI), BF16),
            "dtb": din(f"{s}_dtb", (128, DT_TILES, 1)),
            "A": din(f"{s}_A", (128, DT_TILES, N)),
            "Dp": din(f"{s}_Dp", (128, DT_TILES, 1)),
            "outw": din(f"{s}_outw", (128, DT_TILES, DM), BF16),
        }
    projw_d = din("projw", (128, 4, DM), BF16)
    projb_d = din("projb", (128, MT, 1))
    lng_d = din("lng", (128, MT, 1))
    lnb_d = din("lnb", (128, MT, 1))
    outT_d = nc.dram_tensor("outT", [DM, TOK], F32, kind="ExternalOutput").ap()

    PAD = KC - 1  # 3
    CONVW = 2 * PAD + L  # padded per-batch row length 518

    with tile.TileContext(nc) as tc:
        from contextlib import ExitStack
        with ExitStack() as ctx:
            wpool = ctx.enter_context(tc.tile_pool(name="wpool", bufs=1))
            pers = ctx.enter_context(tc.tile_pool(name="pers", bufs=1))
            work = ctx.enter_context(tc.tile_pool(name="work", bufs=2))
            rep = ctx.enter_context(tc.tile_pool(name="rep", bufs=2))
            scanw = ctx.enter_context(tc.tile_pool(name="scanw", bufs=2))

            def body():
                # ---- load shared weights ----
                projw_t = wpool.tile([128, 4, DM], BF16, tag="projw", name="projw")
                nc.sync.dma_start(projw_t[:], projw_d[:])
                projb_t = wpool.tile([128, MT, 1], F32, tag="projb", name="projb")
                nc.sync.dma_start(projb_t[:], projb_d[:])
                lng_t = wpool.tile([128, MT, 1], F32, tag="lng", name="lng")
                nc.sync.dma_start(lng_t[:], lng_d[:])
                lnb_t = wpool.tile([128, MT, 1], F32, tag="lnb", name="lnb")
                nc.sync.dma_start(lnb_t[:], lnb_d[:])

                xtp_ctx = tc.tile_pool(name="xtp", bufs=1)
                xtp = xtp_ctx.__enter__()
                xT = []
                for m in range(MT):
                    t = xtp.tile([128, TOK], F32, tag=f"xT{m}", name=f"xT{m}")
                    nc.sync.dma_start(t[:], xT_d[m * 128:(m + 1) * 128, :])
                    xT.append(t)

                # ---- shared RMSNorm: xn = x * rsqrt(mean(x^2) + eps) ----
                xn = []
                with tc.tile_pool(name="prms", bufs=1, space="PSUM") as prms:
                    ones_col = wpool.tile([128, 1], F32, tag="ones_col", name="ones_col")
                    nc.vector.memset(ones_col[:], 1.0)
                    ss_ps = prms.tile([1, TOK], F32, tag="ss", name="ss")
                    for fh in range(2):
                        fs = slice(fh * 512, (fh + 1) * 512)
                        for m in range(MT):
                            sq = work.tile([128, 512], F32, tag="sqtmp", name="rms_sq")
                            nc.scalar.square(sq[:], xT[m][:, fs])
                            nc.tensor.matmul(ss_ps[:, fs],ones_col[:],sq[:],
                                             start=(m == 0), stop=(m == MT - 1))
                    # rs = exp(-0.5 * ln(ss/DM + eps))
                    eps1 = wpool.tile([1, 1], F32, tag="eps1", name="eps1")
                    nc.vector.memset(eps1[:], 1e-5)
                    rs_row = work.tile([1, TOK], F32, tag="rowtmp", name="rs_row", bufs=1)
                    nc.scalar.activation(rs_row[:], ss_ps[:], AF.Ln,
                                         scale=1.0 / DM, bias=eps1[:, 0:1])
                    nc.scalar.activation(rs_row[:], rs_row[:], AF.Exp, scale=-0.5)
                    ones1 = wpool.tile([1, 128], F32, tag="ones1", name="ones1")
                    nc.vector.memset(ones1[:], 1.0)
                    rs_ps = prms.tile([128, TOK], F32, tag="rs_rep", name="rs_rep")
                    for fh in range(2):
                        fs = slice(fh * 512, (fh + 1) * 512)
                        nc.tensor.matmul(rs_ps[:, fs],ones1[:],rs_row[:, fs],
                                         start=True, stop=True)
                    for m in range(MT):
                        t = pers.tile([128, TOK], BF16, tag=f"xn{m}", name=f"xn{m}")
                        nc.vector.tensor_mul(t[:], xT[m][:], rs_ps[:])
                        xn.append(t)

                # bf16 copy of x for PE-side residual accumulation
                xTb = []
                for m in range(MT):
                    t = pers.tile([128, TOK], BF16, tag=f"xTb{m}", name=f"xTb{m}")
                    nc.scalar.copy(t[:], xT[m][:])
                    xTb.append(t)
                xtp_ctx.__exit__(None, None, None)

                # ---- per-layer DRAM scratch for B/C row broadcast ----
                dbc_d = {}
                for s2 in ("f", "b"):
                    dbc_d[s2] = nc.dram_tensor(f"{s2}_dbc_scratch", [16, 2, TOK],
                                               BF16, kind="Internal").ap()

                idn = wpool.tile([128, 128], BF16, tag="idn", name="idn")
                from concourse.masks import make_identity
                make_identity(nc, idn[:])

                N_PE_DT = 3

                # ---- proj stage A: in_proj, conv, z-gate (silu table) ----
                def proj_a(s, reverse, ppb):
                    W = lw_d[s]
                    inw_t = wpool.tile([128, 2, 2 * DI], BF16, tag=f"inw{s}", name=f"inw{s}")
                    nc.sync.dma_start(inw_t[:], W["inw"][:])
                    convd_t = wpool.tile([128, DT_TILES * KC, 128], BF16, tag=f"convd{s}", name=f"convd{s}")
                    nc.sync.dma_start(convd_t[:], W["convd"][:])
                    convb_t = wpool.tile([128, DT_TILES, 1], F32, tag=f"convb{s}", name=f"convb{s}")
                    nc.sync.dma_start(convb_t[:], W["convb"][:])

                    xmpad = []
                    sz = []
                    xs = []
                    for dt in range(DT_TILES):
                        t = pers.tile([128, BL, CONVW], BF16, tag=f"xmpad{dt}", name=f"xmpad{dt}")
                        nc.gpsimd.memset(t[:, :, 0:PAD], 0.0)
                        nc.gpsimd.memset(t[:, :, PAD + L:CONVW], 0.0)
                        xmpad.append(t)
                        sz.append(pers.tile([128, TOK], dt_of("SZ"), tag=f"sz{s}{dt}", name=f"sz{s}{dt}"))
                        xs.append(pers.tile([128, TOK], dt_of("XS"), tag=f"xs{s}{dt}", name=f"xs{s}{dt}"))

                    # in_proj: xm tiles first (conv depends on them), z last
                    with tc.tile_pool(name="pp", bufs=ppb, space="PSUM") as pp:
                        for m in range(4):
                            for fh in range(2):
                                fs = slice(fh * 512, (fh + 1) * 512)
                                ps = pp.tile([128, 512], F32, tag="pp", name="pp")
                                for ks in range(2):
                                    nc.tensor.matmul(
                                        ps[:], inw_t[:, ks, m * 128:(m + 1) * 128],
                                        xn[ks][:, fs], start=(ks == 0), stop=(ks == 1))
                                # xm -> padded conv buffer (fh == local batch idx)
                                nc.scalar.copy(xmpad[m][:, fh, PAD:PAD + L], ps[:])

                        # depthwise causal conv + silu
                        for dt in range(DT_TILES):
                            for b in range(BL):
                                ps = pp.tile([128, 512], F32, tag="pp", name="pp")
                                for k in range(KC):
                                    off = k if not reverse else (2 * PAD - k)
                                    nc.tensor.matmul(
                                        ps[:], convd_t[:, dt * KC + k, :],
                                        xmpad[dt][:, b, off:off + L],
                                        start=(k == 0), stop=(k == KC - 1))
                                bs = slice(b * L, (b + 1) * L)
                                nc.scalar.activation(xs[dt][:, bs], ps[:], AF.Silu,
                                                     bias=convb_t[:, dt, 0:1])

                        # z gate tiles (not on the critical chain to the scan)
                        for m in range(4, 8):
                            for fh in range(2):
                                fs = slice(fh * 512, (fh + 1) * 512)
                                ps = pp.tile([128, 512], F32, tag="pp", name="pp")
                                for ks in range(2):
                                    nc.tensor.matmul(
                                        ps[:], inw_t[:, ks, m * 128:(m + 1) * 128],
                                        xn[ks][:, fs], start=(ks == 0), stop=(ks == 1))
                                nc.scalar.activation(sz[m - 4][:, fs], ps[:], AF.Silu)
                    return dict(xs=xs, sz=sz)

                # ---- proj stage B: xproj + dt_proj (exp/ln table only) ----
                # Generator: yields between chunks so it can be interleaved
                # into the other layer's scan loop.
                def proj_b_gen(s, reverse, st):
                    W = lw_d[s]
                    xpw_t = wpool.tile([128, DT_TILES, 96], BF16, tag=f"xpw{s}", name=f"xpw{s}")
                    nc.sync.dma_start(xpw_t[:], W["xpw"][:])
                    dtw_t = wpool.tile([16, DI], BF16, tag=f"dtw{s}", name=f"dtw{s}")
                    nc.sync.dma_start(dtw_t[:], W["dtw"][:])
                    dtb_t = wpool.tile([128, DT_TILES, 1], F32, tag=f"dtb{s}", name=f"dtb{s}")
                    nc.sync.dma_start(dtb_t[:], W["dtb"][:])
                    xs = st["xs"]

                    # xproj -> delta_raw / Brows / Crows; rows to DRAM for bcast
                    dbc = pers.tile([16, 2, TOK], BF16, tag="dbc", name=f"dbc{s}")
                    draw_t = pers.tile([16, TOK], BF16, tag="draw", name=f"draw{s}")
                    draw = draw_t[:, :]
                    with tc.tile_pool(name="pxp", bufs=1, space="PSUM") as pxp:
                        psx = pxp.tile([96, TOK], F32, tag="pxp", name="pxp")
                        for fh in range(2):
                            fs = slice(fh * 512, (fh + 1) * 512)
                            for ks in range(DT_TILES):
                                nc.tensor.matmul(psx[:, fs], xpw_t[:, ks, :], xs[ks][:, fs],
                                                 start=(ks == 0), stop=(ks == DT_TILES - 1))
                            yield
                        nc.scalar.copy(draw, psx[0:16, :])
                        nc.scalar.copy(dbc[:, 0, :], psx[32:48, :])
                        nc.scalar.copy(dbc[:, 1, :], psx[64:80, :])
                    nc.sync.dma_start(dbc_d[s][:], dbc[:])
                    yield

                    # dt_proj + softplus -> delta; w = delta * xs
                    delta = []
                    w_t = []
                    with tc.tile_pool(name="pdt", bufs=2, space="PSUM") as pdt:
                        for dt in range(DT_TILES):
                            dl = pers.tile([128, TOK], dt_of("DELTA"), tag=f"delta{s}{dt}", name=f"delta{s}{dt}")
                            for fh in range(2):
                                fs = slice(fh * 512, (fh + 1) * 512)
                                ps = pdt.tile([128, 512], F32, tag="pdt", name="pdt")
                                nc.tensor.matmul(ps[:], dtw_t[:, dt * 128:(dt + 1) * 128],
                                                 draw[:, fs], start=True, stop=True)
                                e = work.tile([128, 512], F32, tag="detag", name="de")
                                nc.scalar.activation(e[:], ps[:], AF.Exp,
                                                     bias=dtb_t[:, dt, 0:1])
                                nc.scalar.activation(dl[:, fs], e[:], AF.Ln, bias=1.0)
                            delta.append(dl)
                            wt = pers.tile([128, TOK], dt_of("W"), tag=f"w{s}{dt}", name=f"w{s}{dt}")
                            if cfg["WMUL"] == "pool":
                                nc.gpsimd.tensor_mul(wt[:], dl[:], xs[dt][:])
                            else:
                                nc.vector.tensor_mul(wt[:], dl[:], xs[dt][:])
                            w_t.append(wt)
                            if cfg["FUSE_SCAN"]:
                                # poison delta at each sequence start (in scan
                                # order) so exp(delta*A) underflows to zero and
                                # the state resets inside one full-width scan.
                                # (A < 0 strictly, so +1e30 * A -> exp -> 0.)
                                for zc in ((L, 0) if not reverse else (L - 1, 2 * L - 1)):
                                    nc.gpsimd.memset(dl[:, zc:zc + 1], 1e30)
                            yield
                    st["delta"] = delta
                    st["w"] = w_t

                # ---- selective scan + gate + out_proj ----
                def scan_phase(s, reverse, st, y_ps, y_ps3=None, co=None):
                    W = lw_d[s]
                    A_t = wpool.tile([128, DT_TILES, N], F32, tag=f"A{s}", name=f"A{s}")
                    nc.sync.dma_start(A_t[:], W["A"][:])
                    Dp_t = wpool.tile([128, DT_TILES, 1], F32, tag=f"Dp{s}", name=f"Dp{s}")
                    nc.sync.dma_start(Dp_t[:], W["Dp"][:])
                    outw_t = wpool.tile([128, DT_TILES, DM], BF16, tag=f"outw{s}", name=f"outw{s}")
                    nc.sync.dma_start(outw_t[:], W["outw"][:])
                    xs, sz, delta, w_t = st["xs"], st["sz"], st["delta"], st["w"]

                    y_acc3 = pers.tile([128, TOK], dt_of("YACC"), tag=f"yacc{s}", name=f"yacc{s}")
                    pool_bx = cfg["POOL_BX"]
                    pool_p = cfg["POOL_P"]
                    sbufs = cfg["SCANW_BUFS"]
                    n_pe = N_PE_DT if y_ps3 is None else DT_TILES
                    for n in range(N):
                        if co is not None:
                            next(co, None)
                        B_t = rep.tile([128, TOK], dt_of("REP"), tag="B_rep", name="B_rep", bufs=cfg["REP_BUFS"])
                        nc.sync.dma_start(B_t[:], dbc_d[s][n:n + 1, 0, :].broadcast_to([128, TOK]))
                        C_t = rep.tile([128, TOK], dt_of("REP"), tag="C_rep", name="C_rep", bufs=cfg["REP_BUFS"])
                        # C on the ACT-triggered HWDGE queue: halves per-queue load
                        nc.scalar.dma_start(C_t[:], dbc_d[s][n:n + 1, 1, :].broadcast_to([128, TOK]))
                        B_rep = B_t[:, :]
                        C_rep = C_t[:, :]
                        for dt in range(DT_TILES):
                            dA = scanw.tile([128, TOK], dt_of("DA"), tag="dA", name="dA", bufs=cfg["DA_BUFS"])
                            nc.scalar.activation(dA[:], delta[dt][:], AF.Exp,
                                                 scale=A_t[:, dt, n:n + 1])
                            bx = scanw.tile([128, TOK], dt_of("W"), tag="bx", name="bx", bufs=sbufs)
                            if dt in pool_bx:
                                nc.gpsimd.tensor_mul(bx[:], w_t[dt][:], B_rep)
                            elif cfg["SPLIT_MUL"]:
                                for fh in range(2):
                                    fs = slice(fh * 512, (fh + 1) * 512)
                                    nc.vector.tensor_mul(bx[:, fs], w_t[dt][:, fs], B_rep[:, fs])
                            else:
                                nc.vector.tensor_mul(bx[:], w_t[dt][:], B_rep)
                            h = scanw.tile([128, TOK], dt_of("H"), tag="h", name="h", bufs=sbufs)
                            if cfg["FUSE_SCAN"]:
                                if not reverse:
                                    nc.vector.tensor_tensor_scan(
                                        h[:, :], dA[:, :], bx[:, :], 0.0,
                                        ALU.mult, ALU.add)
                                else:
                                    nc.vector.tensor_tensor_scan(
                                        h[:, :][:, ::-1], dA[:, :][:, ::-1],
                                        bx[:, :][:, ::-1], 0.0,
                                        ALU.mult, ALU.add)
                            elif cfg["SPLIT_SCAN"]:
                                HL = L // 2
                                for b in range(BL):
                                    o = b * L
                                    c0 = slice(o, o + HL)
                                    c1 = slice(o + HL, o + L)
                                    if not reverse:
                                        nc.vector.tensor_tensor_scan(
                                            h[:, c0], dA[:, c0], bx[:, c0], 0.0,
                                            ALU.mult, ALU.add)
                                        nc.vector.tensor_tensor_scan(
                                            h[:, c1], dA[:, c1], bx[:, c1],
                                            h[:, o + HL - 1:o + HL],
                                            ALU.mult, ALU.add)
                                    else:
                                        nc.vector.tensor_tensor_scan(
                                            h[:, c1][:, ::-1], dA[:, c1][:, ::-1],
                                            bx[:, c1][:, ::-1], 0.0,
                                            ALU.mult, ALU.add)
                                        nc.vector.tensor_tensor_scan(
                                            h[:, c0][:, ::-1], dA[:, c0][:, ::-1],
                                            bx[:, c0][:, ::-1],
                                            h[:, o + HL:o + HL + 1],
                                            ALU.mult, ALU.add)
                            else:
                                for b in range(BL):
                                    bs = slice(b * L, (b + 1) * L)
                                    if not reverse:
                                        nc.vector.tensor_tensor_scan(
                                            h[:, bs], dA[:, bs], bx[:, bs], 0.0,
                                            ALU.mult, ALU.add)
                                    else:
                                        # reversed-time scan, output written back
                                        # in forward token order
                                        nc.vector.tensor_tensor_scan(
                                            h[:, bs][:, ::-1], dA[:, bs][:, ::-1],
                                            bx[:, bs][:, ::-1], 0.0,
                                            ALU.mult, ALU.add)
                            if dt < n_pe or n > 0:
                                p = scanw.tile([128, TOK], dt_of("P"), tag="p", name="p", bufs=sbufs)
                                if dt in pool_p:
                                    nc.gpsimd.tensor_mul(p[:], h[:], C_rep)
                                elif cfg["SPLIT_MUL"]:
                                    for fh in range(2):
                                        fs = slice(fh * 512, (fh + 1) * 512)
                                        nc.vector.tensor_mul(p[:, fs], h[:, fs], C_rep[:, fs])
                                else:
                                    nc.vector.tensor_mul(p[:], h[:], C_rep)
                                if dt < n_pe:
                                    yp = y_ps[dt] if dt < N_PE_DT else y_ps3
                                    for fh in range(2):
                                        fs = slice(fh * 512, (fh + 1) * 512)
                                        nc.tensor.matmul(yp[:, fs], idn[:], p[:, fs],
                                                         start=(n == 0), stop=(n == N - 1))
                                elif cfg["ADDS4"] == "pool":
                                    nc.gpsimd.tensor_add(y_acc3[:], y_acc3[:], p[:])
                                elif cfg["SPLIT_MUL"]:
                                    for fh in range(2):
                                        fs = slice(fh * 512, (fh + 1) * 512)
                                        nc.vector.tensor_add(y_acc3[:, fs], y_acc3[:, fs], p[:, fs])
                                else:
                                    nc.vector.tensor_add(y_acc3[:], y_acc3[:], p[:])
                            else:
                                # dt == 3, n == 0: initialize y_acc3 = h * C
                                if cfg["SPLIT_MUL"]:
                                    for fh in range(2):
                                        fs = slice(fh * 512, (fh + 1) * 512)
                                        nc.vector.tensor_mul(y_acc3[:, fs], h[:, fs], C_rep[:, fs])
                                else:
                                    nc.vector.tensor_mul(y_acc3[:], h[:], C_rep)

                    # gate: y = y_acc + Dp*xs, then * silu(z) -- in place on xs
                    g = xs
                    for dt in range(DT_TILES):
                        ysrc = (y_ps[dt] if dt < N_PE_DT else
                                (y_ps3 if y_ps3 is not None else y_acc3))
                        if cfg["SPLIT_MUL"]:
                            for fh in range(2):
                                fs = slice(fh * 512, (fh + 1) * 512)
                                nc.vector.scalar_tensor_tensor(
                                    xs[dt][:, fs], xs[dt][:, fs], Dp_t[:, dt, 0:1],
                                    ysrc[:, fs], ALU.mult, ALU.add)
                                nc.vector.tensor_mul(xs[dt][:, fs], xs[dt][:, fs], sz[dt][:, fs])
                        else:
                            nc.vector.scalar_tensor_tensor(
                                xs[dt][:], xs[dt][:], Dp_t[:, dt, 0:1], ysrc[:],
                                ALU.mult, ALU.add)
                            nc.vector.tensor_mul(xs[dt][:], xs[dt][:], sz[dt][:])
                    return g, outw_t

                # ---- out_proj + residual (residual accumulated on PE) ----
                def out_proj(s, g, outw_t):
                    xout = []
                    with tc.tile_pool(name="po", bufs=2, space="PSUM") as po:
                        for m in range(MT):
                            t = pers.tile([128, TOK], BF16, tag=f"x{s}out{m}", name=f"x{s}out{m}")
                            for fh in range(2):
                                fs = slice(fh * 512, (fh + 1) * 512)
                                ps = po.tile([128, 512], F32, tag="po", name="po")
                                for ks in range(DT_TILES):
                                    nc.tensor.matmul(
                                        ps[:], outw_t[:, ks, m * 128:(m + 1) * 128],
                                        g[ks][:, fs], start=(ks == 0), stop=False)
                                nc.tensor.matmul(ps[:], idn[:], xTb[m][:, fs],
                                                 start=False, stop=True)
                                nc.scalar.copy(t[:, fs], ps[:])
                            xout.append(t)
                    return xout

                stf = proj_a("f", reverse=False, ppb=4)
                stb = proj_a("b", reverse=True, ppb=4)
                # scan-phase y accumulators claim 6 PSUM banks; the interleaved
                # stage-B pools for layer b fit in the remaining 2.
                pyac_ctx = tc.tile_pool(name="pyac", bufs=1, space="PSUM")
                pyac = pyac_ctx.__enter__()
                y_ps = [pyac.tile([128, TOK], F32, tag=f"yps{dt}", name=f"yps{dt}")
                        for dt in range(N_PE_DT)]
                for _ in proj_b_gen("f", reverse=False, st=stf):
                    pass
                cob = proj_b_gen("b", reverse=True, st=stb)
                g1, outw_f = scan_phase("f", reverse=False, st=stf, y_ps=y_ps, co=cob)
                for _ in cob:   # drain any remaining stage-B work of layer b
                    pass
                x1 = out_proj("f", g1, outw_f)
                # layer b scan runs with all four d-tiles accumulated on PE:
                # the proj pools are closed by now, freeing the last 2 banks.
                pyac2_ctx = tc.tile_pool(name="pyac2", bufs=1, space="PSUM")
                pyac2 = pyac2_ctx.__enter__()
                y_ps3 = pyac2.tile([128, TOK], F32, tag="yps3", name="yps3")
                g2, outw_b = scan_phase("b", reverse=True, st=stb, y_ps=y_ps, y_ps3=y_ps3)
                pyac2_ctx.__exit__(None, None, None)
                pyac_ctx.__exit__(None, None, None)
                x2 = out_proj("b", g2, outw_b)

                # ---- head: relu(cat(x1,x2) @ proj_w + proj_b), residual, layernorm ----
                cat = x1 + x2
                xn2 = []
                with tc.tile_pool(name="ph", bufs=3, space="PSUM") as ph:
                    for m in range(MT):
                        x2n = pers.tile([128, TOK], F32, tag=f"xn2_{m}", name=f"xn2_{m}")
                        for fh in range(2):
                            fs = slice(fh * 512, (fh + 1) * 512)
                            ps = ph.tile([128, 512], F32, tag="ph", name="ph")
                            for ks in range(4):
                                nc.tensor.matmul(
                                    ps[:],projw_t[:, ks, m * 128:(m + 1) * 128],cat[ks][:, fs], start=(ks == 0), stop=(ks == 3))
                            t = work.tile([128, 512], F32, tag="yh", name="yh")
                            nc.scalar.activation(t[:], ps[:], AF.Relu,
                                                 bias=projb_t[:, m, 0:1])
                            nc.vector.tensor_add(x2n[:, fs], t[:], xTb[m][:, fs])
                        xn2.append(x2n)

                with tc.tile_pool(name="pln", bufs=1, space="PSUM") as pln:
                    ones_col = wpool.tile([128, 1], F32, tag="ones_col2", name="ones_col2")
                    nc.vector.memset(ones_col[:], 1.0)
                    ones1 = wpool.tile([1, 128], F32, tag="ones1b", name="ones1b")
                    nc.vector.memset(ones1[:], 1.0)
                    mu_ps = pln.tile([1, TOK], F32, tag="mu", name="mu")
                    ss_ps = pln.tile([1, TOK], F32, tag="ss2", name="ss2")
                    for fh in range(2):
                        fs = slice(fh * 512, (fh + 1) * 512)
                        for m in range(MT):
                            nc.tensor.matmul(mu_ps[:, fs],ones_col[:],xn2[m][:, fs],
                                             start=(m == 0), stop=(m == MT - 1))
                            sq = work.tile([128, 512], F32, tag="sqtmp", name="ln_sq")
                            nc.scalar.square(sq[:], xn2[m][:, fs])
                            nc.tensor.matmul(ss_ps[:, fs],ones_col[:],sq[:],
                                             start=(m == 0), stop=(m == MT - 1))
                    mu_row = wpool.tile([1, TOK], F32, tag="mu_row", name="mu_row")
                    nc.scalar.mul(mu_row[:], mu_ps[:], 1.0 / DM)
                    # var = ss/DM - mu^2 (built in rstd_row, then rstd in place)
                    rstd_row = wpool.tile([1, TOK], F32, tag="rstd_row", name="rstd_row")
                    nc.scalar.mul(rstd_row[:], ss_ps[:], 1.0 / DM)
                    mu2 = work.tile([1, TOK], F32, tag="rowtmp", name="mu2", bufs=1)
                    nc.vector.tensor_mul(mu2[:], mu_row[:], mu_row[:])
                    nc.vector.tensor_sub(rstd_row[:], rstd_row[:], mu2[:])
                    eps2 = wpool.tile([1, 1], F32, tag="eps2", name="eps2")
                    nc.vector.memset(eps2[:], 1e-5)
                    nc.scalar.activation(rstd_row[:], rstd_row[:], AF.Ln, bias=eps2[:, 0:1])
                    nc.scalar.activation(rstd_row[:], rstd_row[:], AF.Exp, scale=-0.5)
                    mu_rep = pln.tile([128, TOK], F32, tag="mu_rep", name="mu_rep")
                    rs_rep = pln.tile([128, TOK], F32, tag="rs_rep2", name="rs_rep2")
                    for fh in range(2):
                        fs = slice(fh * 512, (fh + 1) * 512)
                        nc.tensor.matmul(mu_rep[:, fs],ones1[:],mu_row[:, fs],
                                         start=True, stop=True)
                        nc.tensor.matmul(rs_rep[:, fs],ones1[:],rstd_row[:, fs],
                                         start=True, stop=True)
                    for m in range(MT):
                        nc.vector.tensor_sub(xn2[m][:], xn2[m][:], mu_rep[:])
                        nc.vector.tensor_mul(xn2[m][:], xn2[m][:], rs_rep[:])
                        nc.scalar.activation(xn2[m][:], xn2[m][:], AF.Identity,
                                             bias=lnb_t[:, m, 0:1],
                                             scale=lng_t[:, m, 0:1])
                        nc.sync.dma_start(outT_d[m * 128:(m + 1) * 128, :], xn2[m][:])

            if loop_k > 1:
                with tc.For_i(0, loop_k, 1):
                    body()
            else:
                body()

    nc.compile()
    _BUILD_CACHE[key] = nc
    return nc


# ======================================================================
# host entry
# ======================================================================

def _make_in_maps(inputs):
    x = np.asarray(inputs["x"], F32_np)
    fw = _prep_layer_weights(inputs["fm_in"], inputs["fm_convw"], inputs["fm_convb"],
                             inputs["fm_xproj"], inputs["fm_dtw"], inputs["fm_dtb"],
                             inputs["fm_Alog"], inputs["fm_D"], inputs["fm_out"],
                             inputs["fm_norm"])
    bw = _prep_layer_weights(inputs["bm_in"], inputs["bm_convw"], inputs["bm_convb"],
                             inputs["bm_xproj"], inputs["bm_dtw"], inputs["bm_dtb"],
                             inputs["bm_Alog"], inputs["bm_D"], inputs["bm_out"],
                             inputs["bm_norm"])
    sh = _prep_shared_weights(inputs["proj_w"], inputs["proj_b"],
                              inputs["ln_g"], inputs["ln_b"])
    base = {}
    for s, w in (("f", fw), ("b", bw)):
        for k, v in w.items():
            if k in ("convb", "inw", "convd", "xpw", "dtw", "dtb", "A", "Dp",
                     "outw"):
                base[f"{s}_{k}"] = v
    base["projw"] = sh["projw"]
    base["projb"] = sh["projb"]
    base["lng"] = sh["lng"]
    base["lnb"] = sh["lnb"]

    in_maps = []
    for c in range(NCORES):
        xc = x[c * BL:(c + 1) * BL]                       # (BL, L, DM)
        xTc = np.ascontiguousarray(xc.reshape(TOK, DM).T)  # (DM, TOK)
        m = dict(base)
        m["xT"] = xTc
        in_maps.append(m)
    return in_maps


def _unshard(results):
    outs = []
    for c in range(NCORES):
        oT = results[c]["outT"]                            # (DM, TOK)
        outs.append(np.ascontiguousarray(oT.T.reshape(BL, L, DM)))
    return np.concatenate(outs, axis=0).astype(F32_np)


def kernel(**inputs):
    from concourse import bass_utils
    nc = _build(loop_k=1)
    in_maps = _make_in_maps(inputs)
    res = bass_utils.run_bass_kernel_spmd(nc, in_maps, core_ids=list(range(NCORES)))
    return _unshard(res.results)

